# revision 1
# baseline (speedup 1.0000x reference)
"""AttentionBlock Trainium2 kernel (B=4, N=2048, C=1024, H=16, D=64, EMB=1024).

    se = emb @ W_emb.T + b_emb;  scale, shift = split(se, 2, -1)
    h  = LN(x) * (1+scale) + shift
    q,k,v = split(h @ W_proj.T) -> (B,H,N,D);  q = LN(q); k = LN(k)  (over D)
    o  = softmax(q k^T / sqrt(D)) v  -> (B,N,C)
    out = o + o @ W_out.T

Sharding: 8 cores; core c -> batch b=c//2, query-half j=c%2. The host rolls
the token axis per core so its query tokens are always tokens 0:1024
(attention is permutation-equivariant over key/value tokens), giving one
symmetric SPMD NEFF with no collectives. Each core computes the full-batch
preamble (se/h/k/v over all 2048 tokens), and q/attention/out-proj for its
1024 rows.

Dataflow is feature-major (channels on partitions) end to end:
  - LayerNorm over channels == partition reduction -> ones-column matmuls.
  - Per-token (free-dim) scalars broadcast across partitions by bouncing a
    row through DRAM (DRAM APs allow step-0 partition dims; SBUF APs don't).
  - q/k LN centering rides the score matmul as an augmented 65th row
    (k_aug row64 = 8*mu_k[m], q_aug row64 = -8*mu_q[n]*rq[n]); the rk[m]/8
    factor is applied by the ACT exp per-partition scale operand.
  - Softmax denominators come free as a ones column appended to v; the
    division is deferred until after the attn@v matmul.
  - The residual is folded into the output projection: W_res = (I+W_out).T.

Matmuls run in float32r (1 cycle/row at free-dim>=256, ~1.5e-4 rel err);
attention probabilities/values use bf16 (errors cancel in the softmax ratio).
"""

import sys

sys.path.insert(0, "/opt/trn_rl_repo")

import numpy as np

import concourse.bass as bass
import concourse.mybir as mybir
import concourse.tile as tile
from concourse import bacc
from concourse.bass_utils import run_bass_kernel_spmd

P = 128
B, N, C = 4, 2048, 1024
H, D = 16, 64
EMB = 1024
EPS = 1e-5
T = N          # tokens per batch on each core (k/v coverage)
TQ = N // 2    # query tokens per core
CH = C // P    # 8 channel chunks
O2 = 2 * C
NCORES = 8
TT = 256       # token tile in phase A1
NTT = T // TT
NMT = T // 512   # 4  key-token tiles (512)
NMC = T // P     # 16 key-token chunks (128)
NNT = TQ // 512  # 2  query-token tiles (512)

F32 = mybir.dt.float32
F32R = mybir.dt.float32r
BF16 = mybir.dt.bfloat16
MUL = mybir.AluOpType.mult
ADD = mybir.AluOpType.add
SUB = mybir.AluOpType.subtract
EXP = mybir.ActivationFunctionType.Exp
SQRT = mybir.ActivationFunctionType.Sqrt

_cached = {}


def build_kernel(debug=None):
    nc = bacc.Bacc()

    xT = nc.dram_tensor("xT", [C, T], F32R, kind="ExternalInput")
    embT = nc.dram_tensor("embT", [EMB, T], F32R, kind="ExternalInput")
    WembT = nc.dram_tensor("WembT", [EMB, O2], F32R, kind="ExternalInput")
    bemb = nc.dram_tensor("bemb", [P, O2 // P], F32, kind="ExternalInput")
    WprojT = nc.dram_tensor("WprojT", [C, 3 * C], F32R, kind="ExternalInput")
    WresT = nc.dram_tensor("WresT", [C, C], F32R, kind="ExternalInput")
    out = nc.dram_tensor("out", [TQ, C], F32, kind="ExternalOutput")

    xT_r = xT.rearrange("(ch p) t -> p ch t", p=P)
    embT_r = embT.rearrange("(ch p) t -> p ch t", p=P)
    WembT_r = WembT.rearrange("(ch p) o -> p ch o", p=P)
    WprojT_r = WprojT.rearrange("(ch p) o -> p ch o", p=P)
    WresT_r = WresT.rearrange("(ch p) o -> p ch o", p=P)

    with tile.TileContext(nc) as tc:
        with (
            tc.tile_pool(name="const", bufs=1) as const,
            tc.tile_pool(name="main", bufs=1) as main,
            tc.tile_pool(name="dram", bufs=2, space="DRAM") as dram,
            tc.tile_pool(name="ps_mm", bufs=3, space="PSUM") as ps_mm,
            tc.tile_pool(name="ps_ot", bufs=2, space="PSUM") as ps_ot,
            tc.tile_pool(name="ps_row", bufs=3, space="PSUM") as ps_row,
        ):
            # ---------------- constants ----------------
            eps_t = const.tile([P, 1], F32, name="eps_t")
            nc.vector.memset(eps_t[:], EPS)
            # memset can't emit float32r: stage constants in F32, copy-round.
            cscr = const.tile([P, 65], F32, name="cscr")
            ones_col = const.tile([P, 1], F32R, name="ones_col")
            nc.vector.memset(cscr[:, 0:1], 1.0)
            nc.vector.tensor_copy(ones_col[:], cscr[:, 0:1])
            # per-head partition-block sums: lhsT cols 0 and 64 select heads,
            # so the row-matmul output lands at partitions 0 and 64 (aligned).
            bo8 = const.tile([P, 65], F32R, name="bo8")      # +1/8
            bon8 = const.tile([P, 65], F32R, name="bon8")    # -1/8
            bo64 = const.tile([P, 65], F32R, name="bo64")    # +1/64
            for t_, v_ in ((bo8, 0.125), (bon8, -0.125), (bo64, 1.0 / 64)):
                nc.vector.memset(cscr[:], 0.0)
                nc.vector.memset(cscr[0:64, 0:1], v_)
                nc.vector.memset(cscr[64:128, 64:65], v_)
                nc.vector.tensor_copy(t_[:], cscr[:])
            bemb_sb = const.tile([P, O2 // P], F32, name="bemb_sb")
            nc.sync.dma_start(bemb_sb[:], bemb[:])

            h_sb = main.tile([P, CH, T], F32R, name="h_sb")  # 64KB/part
            o_fm = main.tile([P, CH, TQ], F32R, name="o_fm")  # 32KB/part

            # ============ Phase A1: se + LN(x) + FiLM -> h ============
            with (
                tc.tile_pool(name="wembp", bufs=1) as wembp,
                tc.tile_pool(name="a1s", bufs=2) as a1s,
                tc.tile_pool(name="a1r", bufs=2) as a1r,
            ):
                wemb_sb = wembp.tile([P, CH, O2], F32R, name="wemb_sb")
                nc.sync.dma_start(wemb_sb[:], WembT_r)

                for tt in range(NTT):
                    tsl = slice(tt * TT, (tt + 1) * TT)
                    x_t = a1s.tile([P, CH, TT], F32R, name="x_t")
                    nc.sync.dma_start(x_t[:], xT_r[:, :, tsl])
                    e_t = a1s.tile([P, CH, TT], F32R, name="e_t")
                    nc.sync.dma_start(e_t[:], embT_r[:, :, tsl])

                    # LN stats over channels (partition reduction via matmul)
                    ps_s = ps_row.tile([2, 512], F32, name="ps_s", tag="ps_row")
                    ps_s2 = ps_row.tile([2, 512], F32, name="ps_s2", tag="ps_row")
                    for ch in range(CH):
                        x2c = a1r.tile([P, TT], F32R, name="x2c", tag="scr")
                        nc.scalar.square(x2c[:], x_t[:, ch, :])
                        nc.tensor.matmul(ps_s[0:1, 0:TT], ones_col[:], x_t[:, ch, :],
                                         start=(ch == 0), stop=(ch == CH - 1))
                        nc.tensor.matmul(ps_s2[0:1, 0:TT], ones_col[:], x2c[:],
                                         start=(ch == 0), stop=(ch == CH - 1))
                    mu = a1r.tile([1, TT], F32, name="mu")
                    m2 = a1r.tile([1, TT], F32, name="m2")
                    vr = a1r.tile([1, TT], F32, name="vr")
                    nmr = a1r.tile([1, TT], F32, name="nmr")
                    nc.vector.tensor_scalar_mul(mu[:], ps_s[0:1, 0:TT], 1.0 / C)
                    nc.vector.tensor_tensor(m2[:], mu[:], mu[:], MUL)  # mu^2
                    # var = ps_s2/C - mu^2  (one input in PSUM, one SBUF)
                    nc.vector.scalar_tensor_tensor(vr[:], ps_s2[0:1, 0:TT], 1.0 / C, m2[:],
                                                   mybir.AluOpType.mult, SUB)
                    nc.scalar.activation(vr[:], vr[:], SQRT, bias=eps_t[0:1], scale=1.0)
                    nc.vector.reciprocal(vr[:], vr[:])          # rstd
                    # nmr = -mu * rstd
                    nc.vector.scalar_tensor_tensor(nmr[:], mu[:], -1.0, vr[:], MUL, MUL)
                    rstd = vr[:]

                    rows_d = dram.tile([2, TT], F32, name="rows_d")
                    nc.sync.dma_start(rows_d[0:1, :], rstd)
                    nc.sync.dma_start(rows_d[1:2, :], nmr[:])
                    rstd_bc = a1r.tile([P, TT], F32, name="rstd_bc")
                    nc.sync.dma_start(rstd_bc[:], rows_d[0:1, :].to_broadcast((P, TT)))
                    nmr_bc = a1r.tile([P, TT], F32, name="nmr_bc")
                    nc.sync.dma_start(nmr_bc[:], rows_d[1:2, :].to_broadcast((P, TT)))

                    for ch in range(CH):
                        ps_sc = ps_mm.tile([P, 512], F32, name="ps_sc", tag="ps_mm")
                        for ech in range(CH):
                            nc.tensor.matmul(ps_sc[:, 0:TT],
                                             wemb_sb[:, ech, ch * P:(ch + 1) * P],
                                             e_t[:, ech, :],
                                             start=(ech == 0), stop=(ech == CH - 1))
                        ps_sh = ps_mm.tile([P, 512], F32, name="ps_sh", tag="ps_mm")
                        for ech in range(CH):
                            nc.tensor.matmul(ps_sh[:, 0:TT],
                                             wemb_sb[:, ech, C + ch * P:C + (ch + 1) * P],
                                             e_t[:, ech, :],
                                             start=(ech == 0), stop=(ech == CH - 1))
                        nc.vector.tensor_scalar_add(ps_sc[:, 0:TT], ps_sc[:, 0:TT], bemb_sb[:, ch:ch + 1])
                        nc.vector.tensor_scalar_add(ps_sh[:, 0:TT], ps_sh[:, 0:TT], bemb_sb[:, CH + ch:CH + ch + 1])
                        xn = a1r.tile([P, TT], F32, name="xn", tag="scr")
                        nc.vector.tensor_tensor(xn[:], x_t[:, ch, :], rstd_bc[:], MUL)
                        nc.vector.tensor_tensor(xn[:], xn[:], nmr_bc[:], ADD)
                        nc.vector.tensor_tensor(xn[:], xn[:], ps_sc[:, 0:TT], MUL)
                        nc.vector.tensor_tensor(h_sb[:, ch, tsl], xn[:], ps_sh[:, 0:TT], ADD)

            if debug == "h":
                # dump h (first TQ tokens) to out: out[t, c] = h[c, t]
                # dump h feature-major: out viewed as [C, TQ]
                nc.gpsimd.dma_start(out.rearrange("(ch p) t -> p ch t", p=P),
                                    h_sb[:, :, 0:TQ])
            if debug != "h":
                # ============ Phase B: per-head-pair qkv + attention ============
                with (
                    tc.tile_pool(name="bw", bufs=1) as bw,
                    tc.tile_pool(name="batt", bufs=1) as batt,
                tc.tile_pool(name="bp", bufs=2) as bp,
                    tc.tile_pool(name="bsm", bufs=1) as bsm,
                    tc.tile_pool(name="bq", bufs=2) as bq,
                ):
                    _dbg_b = debug in ("ka", "qa", "p", "b1")
                    do_q = debug not in ("ka",)
                    do_sc = debug not in ("ka", "qa")
                    do_o = debug not in ("ka", "qa", "p")
                    for hq in range(1 if _dbg_b else 4):  # head quads
                        wv_sb = bw.tile([P, CH, 256], F32R, name="wv_sb")
                        nc.sync.dma_start(wv_sb[:], WprojT_r[:, :, 2 * C + hq * 256:2 * C + (hq + 1) * 256])
                        v_sb = batt.tile([P, NMC, 4, 72], BF16, name="v_sb")
                        nc.vector.memset(v_sb[:, :, :, 64:65], 1.0)
                        for mtk in range(NMC):
                            ps_v = ps_mm.tile([P, 512], F32, name="ps_v", tag="ps_mm")
                            for ch in range(CH):
                                nc.tensor.matmul(ps_v[:, 0:256], h_sb[:, ch, mtk * P:(mtk + 1) * P],
                                                 wv_sb[:, ch, :], start=(ch == 0), stop=(ch == CH - 1))
                            for hh in range(4):
                                nc.vector.tensor_copy(v_sb[:, mtk, hh, 0:64],
                                                      ps_v[:, hh * 64:(hh + 1) * 64])

                        for hp in ((2 * hq,) if debug in ("ka", "qa", "p") else (2 * hq, 2 * hq + 1)):
                            wqk_sb = bw.tile([P, CH, 256], F32R, name="wqk_sb")
                            nc.sync.dma_start(wqk_sb[:, :, 0:128], WprojT_r[:, :, hp * P:(hp + 1) * P])
                            nc.sync.dma_start(wqk_sb[:, :, 128:256],
                                              WprojT_r[:, :, C + hp * P:C + (hp + 1) * P])

                            # ---- k projection + stats (2 heads on partitions) ----
                            ka0 = batt.tile([65, T], F32R, name="ka0")
                            ka1 = batt.tile([65, T], F32R, name="ka1")
                            sk8_d = dram.tile([2, T], F32, name="sk8_d")
                            ex2k_d = dram.tile([2, T], F32, name="ex2k_d")
                            for mt in range(NMT):
                                msl = slice(mt * 512, (mt + 1) * 512)
                                ps_k = ps_mm.tile([P, 512], F32, name="ps_k", tag="ps_mm")
                                for ch in range(CH):
                                    nc.tensor.matmul(ps_k[:], wqk_sb[:, ch, 128:256],
                                                     h_sb[:, ch, msl],
                                                     start=(ch == 0), stop=(ch == CH - 1))
                                k2sb = bsm.tile([P, 512], F32R, name="k2sb")
                                nc.vector.tensor_copy(k2sb[:], ps_k[:])
                                nc.vector.tensor_copy(ka0[0:64, msl], ps_k[0:64, :])
                                nc.vector.tensor_copy(ka1[0:64, msl], ps_k[64:128, :])
                                ksq = bsm.tile([P, 512], F32R, name="ksq")
                                nc.scalar.square(ksq[:], ps_k[:])
                                ps_kr = ps_row.tile([65, 512], F32, name="ps_kr", tag="ps_row")
                                nc.tensor.matmul(ps_kr[:], bo8[:], k2sb[:], start=True, stop=True)
                                ps_kr2 = ps_row.tile([65, 512], F32, name="ps_kr2", tag="ps_row")
                                nc.tensor.matmul(ps_kr2[:], bo64[:], ksq[:], start=True, stop=True)
                                # k_aug row 64 = 8*mu_k
                                nc.vector.tensor_copy(ka0[64:65, msl], ps_kr[0:1, :])
                                nc.vector.tensor_copy(ka1[64:65, msl], ps_kr[64:65, :])
                                skr = bsm.tile([65, 512], F32, name="skr")
                                nc.vector.tensor_copy(skr[0:1, :], ps_kr[0:1, :])
                                nc.vector.tensor_copy(skr[64:65, :], ps_kr[64:65, :])
                                nc.sync.dma_start(sk8_d[0:1, msl], skr[0:1, :])
                                nc.sync.dma_start(sk8_d[1:2, msl], skr[64:65, :])
                                exr = bsm.tile([65, 512], F32, name="exr")
                                nc.vector.tensor_copy(exr[0:1, :], ps_kr2[0:1, :])
                                nc.vector.tensor_copy(exr[64:65, :], ps_kr2[64:65, :])
                                nc.sync.dma_start(ex2k_d[0:1, msl], exr[0:1, :])
                                nc.sync.dma_start(ex2k_d[1:2, msl], exr[64:65, :])

                            # rk/8 in column form [P, NMC, 2] via DRAM gather
                            sk8T = bsm.tile([P, NMC, 2], F32, name="sk8T")
                            for h_ in range(2):
                                nc.sync.dma_start(sk8T[:, :, h_],
                                                  sk8_d[h_].rearrange("(mc p) -> p mc", p=P))
                            rk8 = bsm.tile([P, NMC, 2], F32, name="rk8")
                            nc.vector.tensor_scalar_mul(rk8[:], sk8T[:], 0.125)   # mu_k
                            nc.vector.tensor_tensor(rk8[:], rk8[:], rk8[:], MUL)  # mu_k^2
                            ex2kT = bsm.tile([P, NMC, 2], F32, name="ex2kT")
                            for h_ in range(2):
                                nc.sync.dma_start(ex2kT[:, :, h_],
                                                  ex2k_d[h_].rearrange("(mc p) -> p mc", p=P))
                            nc.vector.tensor_tensor(rk8[:], ex2kT[:], rk8[:], SUB)
                            nc.scalar.activation(rk8[:], rk8[:], SQRT, bias=eps_t[:], scale=1.0)
                            nc.vector.reciprocal(rk8[:], rk8[:])
                            nc.vector.tensor_scalar_mul(rk8[:], rk8[:], 0.125)    # rk/8

                            if debug == "ka":
                                nc.gpsimd.dma_start(out.rearrange("(a b) t -> a (b t)", a=P)[0:65, 0:T],
                                                    ka0[:])
                                nc.gpsimd.dma_start(out.rearrange("(a b) t -> a (b t)", a=P)[0:P, T:T + NMC * 2],
                                                    rk8[:].rearrange("p a b -> p (a b)"))
                            if not do_q:
                                continue
                            # ---- q projection + stats ----
                            nsq8 = bsm.tile([65, TQ], F32, name="nsq8")
                            ex2q = bsm.tile([65, TQ], F32, name="ex2q")
                            q2a = []
                            for nt in range(NNT):
                                nsl = slice(nt * 512, (nt + 1) * 512)
                                ps_q = ps_mm.tile([P, 512], F32, name="ps_q", tag="ps_mm")
                                for ch in range(CH):
                                    nc.tensor.matmul(ps_q[:], wqk_sb[:, ch, 0:128],
                                                     h_sb[:, ch, nsl],
                                                     start=(ch == 0), stop=(ch == CH - 1))
                                q2t = bq.tile([P, 512], F32R, name="q2t", tag="q2t")
                                nc.vector.tensor_copy(q2t[:], ps_q[:])
                                q2a.append(q2t)
                                qsq = bsm.tile([P, 512], F32R, name="qsq")
                                nc.scalar.square(qsq[:], ps_q[:])
                                ps_qr = ps_row.tile([65, 512], F32, name="ps_qr", tag="ps_row")
                                nc.tensor.matmul(ps_qr[:], bon8[:], q2t[:], start=True, stop=True)
                                ps_qr2 = ps_row.tile([65, 512], F32, name="ps_qr2", tag="ps_row")
                                nc.tensor.matmul(ps_qr2[:], bo64[:], qsq[:], start=True, stop=True)
                                nc.vector.tensor_copy(nsq8[:, nsl], ps_qr[:])
                                nc.vector.tensor_copy(ex2q[:, nsl], ps_qr2[:])

                            rq = bsm.tile([65, TQ], F32, name="rq")
                            nc.vector.tensor_tensor(rq[:], nsq8[:], nsq8[:], MUL)
                            nc.vector.tensor_scalar_mul(rq[:], rq[:], 1.0 / 64)   # mu_q^2
                            nc.vector.tensor_tensor(rq[:], ex2q[:], rq[:], SUB)
                            nc.scalar.activation(rq[:], rq[:], SQRT, bias=eps_t[0:65], scale=1.0)
                            nc.vector.reciprocal(rq[:], rq[:])
                            rq_d = dram.tile([2, TQ], F32, name="rq_d")
                            nc.sync.dma_start(rq_d[0:1, :], rq[0:1, :])
                            nc.sync.dma_start(rq_d[1:2, :], rq[64:65, :])
                            rq_bc = bsm.tile([P, TQ], F32, name="rq_bc")
                            nc.sync.dma_start(rq_bc[0:64, :],
                                              rq_d[0:1, :].to_broadcast((64, TQ)))
                            nc.sync.dma_start(rq_bc[64:128, :],
                                              rq_d[1:2, :].to_broadcast((64, TQ)))

                            qa0 = batt.tile([65, TQ], F32R, name="qa0")
                            qa1 = batt.tile([65, TQ], F32R, name="qa1")
                            for nt in range(NNT):
                                nsl = slice(nt * 512, (nt + 1) * 512)
                                nc.vector.tensor_tensor(qa0[0:64, nsl], q2a[nt][0:64, :],
                                                        rq_bc[0:64, nsl], MUL)
                                nc.vector.tensor_tensor(qa1[0:64, nsl], q2a[nt][64:128, :],
                                                        rq_bc[64:128, nsl], MUL)
                            nc.vector.tensor_tensor(qa0[64:65, :], nsq8[0:1, :], rq[0:1, :], MUL)
                            nc.vector.tensor_tensor(qa1[64:65, :], nsq8[64:65, :], rq[64:65, :], MUL)

                            if debug == "qa":
                                ofl = out.rearrange("(a b) t -> a (b t)", a=P)
                                nc.gpsimd.dma_start(ofl[0:65, 0:TQ], qa0[:])
                                nc.gpsimd.dma_start(ofl[0:P, 2048:2560], q2a[0][:])
                                nc.gpsimd.dma_start(ofl[0:65, 4096:4096 + TQ], rq[:])
                                nc.gpsimd.dma_start(ofl[0:P, 6144:6144 + TQ], rq_bc[:])
                            if not do_sc:
                                continue
                            # ---- scores + exp + o per head ----
                            for hh, (ka, qa) in enumerate(((ka0, qa0), (ka1, qa1))):
                                head = 2 * hp + hh
                                vidx = (hp % 2) * 2 + hh
                                for nt in range(NNT):
                                    nsl = slice(nt * 512, (nt + 1) * 512)
                                    if debug == "p" and not (head == 0 and nt == 0):
                                        continue
                                    ps_o = None
                                    if debug != "p":
                                        ps_o = ps_ot.tile([65, 512], F32, name="ps_o", tag="ps_ot")
                                    for mh in range(2):
                                        p_sb = bp.tile([P, NMC // 2, 512], BF16, name="p_sb")
                                        for mi in range(NMC // 2):
                                            mc = mh * (NMC // 2) + mi
                                            ps_sT = ps_mm.tile([P, 512], F32, name="ps_sT", tag="ps_mm")
                                            nc.tensor.matmul(ps_sT[:], ka[:, mc * P:(mc + 1) * P],
                                                             qa[:, nsl], start=True, stop=True)
                                            nc.scalar.activation(p_sb[:, mi, :], ps_sT[:], EXP,
                                                                 bias=0.0, scale=rk8[:, mc, hh:hh + 1])
                                        if debug == "p":
                                            nc.gpsimd.dma_start(
                                                out.rearrange("(a b) t -> a (b t)", a=P)[:, mh * 4096:(mh + 1) * 4096],
                                                p_sb[:, :, :])
                                            continue
                                        for mi in range(NMC // 2):
                                            mc = mh * (NMC // 2) + mi
                                            nc.tensor.matmul(ps_o[:], v_sb[:, mc, vidx, 0:65],
                                                             p_sb[:, mi, :],
                                                             start=(mc == 0), stop=(mc == NMC - 1))
                                    if debug == "p":
                                        continue
                                    rec = bsm.tile([1, 512], F32, name="rec")
                                    nc.vector.reciprocal(rec[:], ps_o[64:65, :])
                                    rec_d = dram.tile([1, 512], F32, name="rec_d")
                                    nc.sync.dma_start(rec_d[:], rec[:])
                                    rec_bc = bsm.tile([64, 512], F32, name="rec_bc")
                                    nc.sync.dma_start(rec_bc[:], rec_d[:].to_broadcast((64, 512)))
                                    nc.vector.tensor_tensor(
                                        o_fm[(head % 2) * 64:(head % 2) * 64 + 64, head // 2, nsl],
                                        ps_o[0:64, :], rec_bc[:], MUL)

                if debug == "b1":
                    nc.gpsimd.dma_start(out.rearrange("(ch p) t -> p ch t", p=P),
                                        o_fm[:, :, :])
                # ============ Phase C: out = o_fm.T @ (I + W_out).T ============
                if debug is None:
                  with tc.tile_pool(name="cw", bufs=2) as cw:
                    for jt in range(C // 512):
                        wres_sb = cw.tile([P, CH, 512], F32R, name="wres_sb")
                        nc.sync.dma_start(wres_sb[:], WresT_r[:, :, jt * 512:(jt + 1) * 512])
                        for ns in range(TQ // P):
                            ps_f = ps_mm.tile([P, 512], F32, name="ps_f", tag="ps_mm")
                            for cg in range(CH):
                                nc.tensor.matmul(ps_f[:], o_fm[:, cg, ns * P:(ns + 1) * P],
                                                 wres_sb[:, cg, :],
                                                 start=(cg == 0), stop=(cg == CH - 1))
                            f_sb = cw.tile([P, 512], F32, name="f_sb")
                            nc.vector.tensor_copy(f_sb[:], ps_f[:])
                            nc.sync.dma_start(out[ns * P:(ns + 1) * P, jt * 512:(jt + 1) * 512],
                                              f_sb[:])

    nc.finalize()
    return nc


def _prep_host(x, emb, W_emb, b_emb, W_proj, W_out):
    W_embT = np.ascontiguousarray(W_emb.T.astype(np.float32))
    W_projT = np.ascontiguousarray(W_proj.T.astype(np.float32))
    W_resT = np.ascontiguousarray((np.eye(C, dtype=np.float32) + W_out).T.astype(np.float32))
    bemb2 = b_emb.astype(np.float32).copy()
    bemb2[:C] += 1.0                       # fold the FiLM "+1" into the bias
    bemb_col = np.ascontiguousarray(bemb2.reshape(O2 // P, P).T)

    in_maps = []
    for c in range(NCORES):
        b, j = c // 2, c % 2
        perm = np.concatenate([np.arange(j * TQ, (j + 1) * TQ),
                               np.arange((1 - j) * TQ, (2 - j) * TQ)])
        in_maps.append({
            "xT": np.ascontiguousarray(x[b][perm].T.astype(np.float32)),
            "embT": np.ascontiguousarray(emb[b][perm].T.astype(np.float32)),
            "WembT": W_embT, "bemb": bemb_col,
            "WprojT": W_projT, "WresT": W_resT,
        })
    return in_maps


def kernel(x, emb, W_emb, b_emb, W_proj, W_out, _trace=False):
    x = np.asarray(x); emb = np.asarray(emb)
    W_emb = np.asarray(W_emb); b_emb = np.asarray(b_emb)
    W_proj = np.asarray(W_proj); W_out = np.asarray(W_out)

    if "nc" not in _cached:
        _cached["nc"] = build_kernel()
    nc = _cached["nc"]

    in_maps = _prep_host(x, emb, W_emb, b_emb, W_proj, W_out)
    res = run_bass_kernel_spmd(nc, in_maps, core_ids=list(range(NCORES)), trace=_trace)
    _cached["last_result"] = res

    outp = np.empty((B, N, C), dtype=np.float32)
    for c in range(NCORES):
        b, j = c // 2, c % 2
        outp[b, j * TQ:(j + 1) * TQ, :] = res.results[c]["out"]
    return outp



# revision 9
# speedup vs baseline: 1.3925x; 1.3925x over previous
"""AttentionBlock Trainium2 kernel (B=4, N=2048, C=1024, H=16, D=64, EMB=1024).

    se = emb @ W_emb.T + b_emb;  scale, shift = split(se, 2, -1)
    h  = LN(x) * (1+scale) + shift
    q,k,v = split(h @ W_proj.T) -> (B,H,N,D);  q = LN(q); k = LN(k)  (over D)
    o  = softmax(q k^T / sqrt(D)) v  -> (B,N,C)
    out = o + o @ W_out.T

Sharding: 8 cores; core c -> batch b=c//2, token-half j=c%2 (natural order).
Each core computes the preamble (se/h/k/v/stats) only for its OWN 1024
tokens, then token-pair cores exchange k/v/rk via a pair-wise AllGather
(DRAM bounce); the peer block is read back with a register-offset DMA
(peer = 1 - pid%2), keeping one uniform SPMD NEFF. Attention runs in two
passes: pass1 over own key-chunks (overlapping the collective), pass2 over
peer chunks; softmax numerator/denominator partials combine at the end.

Dataflow is feature-major (channels on partitions) end to end:
  - LayerNorm over channels == partition reduction -> ones-column matmuls.
  - Per-token (free-dim) scalars broadcast across partitions by bouncing a
    row through DRAM.
  - q LN-centering folds into q (qa = q*rq - mu_q*rq); k needs NO centering
    because sum_d qa[d] = 0 kills the mu_k cross term exactly.
  - k LN-scaling rides the ACT exp as a per-partition scale operand (rk/8).
  - Scores use 64-row 2-head tile_position row-tiling (2 heads concurrent
    in the PE array); exp is batched [128, 2x512] over both query tiles.
  - Softmax denominators come free as a ones-column appended to v; division
    deferred past attn@v; all rsqrt/reciprocal via exp(-ln x) on ACT so the
    whole kernel uses ONE activation table set (no ~2.7us table switches).
  - The residual is folded into the output projection: W_res = (I+W_out).T.

Matmuls in float32r (1 cycle/row); k/p/v in bf16 (errors largely cancel in
the softmax ratio).
"""

import sys

sys.path.insert(0, "/opt/trn_rl_repo")

import numpy as np

import concourse.bass as bass
import concourse.mybir as mybir
import concourse.tile as tile
from concourse import bacc
from concourse.bass_utils import run_bass_kernel_spmd

P = 128
B, N, C = 4, 2048, 1024
H, D = 16, 64
EMB = 1024
EPS = 1e-5
T = N          # tokens per batch (k/v coverage after exchange)
TQ = N // 2    # own tokens per core
CH = C // P    # 8 channel chunks
O2 = 2 * C
NCORES = 8
NMC = T // P       # 16 key-token 128-chunks (8 own + 8 peer)
NMCH = NMC // 2    # 8
NNT = TQ // 512    # 2 query-token 512-tiles
NMT = TQ // 512    # 2 own 512-tiles for projections

# packed pair-exchange payload, f32 units, 8 slots per partition:
#   [0:512]     k  (bf16 x1024 viewed as f32 x512)    slot = head-pair hp
#   [512:1088]  v  (bf16 [16 heads, 72] as f32 x576)  slot = own key-chunk mc
#   [1088:1104] rk/8 column form [16 heads]           slot = own key-chunk mc
CCW = 1104

F32 = mybir.dt.float32
F32R = mybir.dt.float32r
BF16 = mybir.dt.bfloat16
MUL = mybir.AluOpType.mult
ADD = mybir.AluOpType.add
SUB = mybir.AluOpType.subtract
EXP = mybir.ActivationFunctionType.Exp
LN_ = mybir.ActivationFunctionType.Ln
NL8 = -2.0794415416798357  # -ln(8)

_cached = {}


def build_kernel():
    nc = bacc.Bacc()

    xT = nc.dram_tensor("xT", [C, TQ], F32R, kind="ExternalInput")
    embT = nc.dram_tensor("embT", [EMB, TQ], F32R, kind="ExternalInput")
    WembT = nc.dram_tensor("WembT", [EMB, O2], F32R, kind="ExternalInput")
    bemb = nc.dram_tensor("bemb", [P, O2 // P], F32, kind="ExternalInput")
    WprojT = nc.dram_tensor("WprojT", [C, 3 * C], F32R, kind="ExternalInput")
    WresT = nc.dram_tensor("WresT", [C, C], F32R, kind="ExternalInput")
    out = nc.dram_tensor("out", [TQ, C], F32, kind="ExternalOutput")

    xT_r = xT.rearrange("(ch p) t -> p ch t", p=P)
    embT_r = embT.rearrange("(ch p) t -> p ch t", p=P)
    WembT_r = WembT.rearrange("(ch p) o -> p ch o", p=P)
    WprojT_r = WprojT.rearrange("(ch p) o -> p ch o", p=P)
    WresT_r = WresT.rearrange("(ch p) o -> p ch o", p=P)

    with tile.TileContext(nc) as tc:
        with (
            tc.tile_pool(name="const", bufs=1) as const,
            tc.tile_pool(name="dram", bufs=2, space="DRAM") as dram,
        ):
            # ---------------- constants ----------------
            eps_t = const.tile([P, 1], F32, name="eps_t")
            nc.vector.memset(eps_t[:], EPS)
            nl8_t = const.tile([P, 1], F32, name="nl8_t")
            nc.vector.memset(nl8_t[:], NL8)
            # memset can't emit float32r/bf16: stage constants in F32, copy.
            cscr = const.tile([P, 65], F32, name="cscr")
            ones_r = const.tile([P, 1], F32R, name="ones_r")
            nc.vector.memset(cscr[:, 0:1], 1.0)
            nc.vector.tensor_copy(ones_r[:], cscr[:, 0:1])
            # per-head partition-block sums: lhsT cols 0 and 64 select heads.
            bo8b = const.tile([P, 65], BF16, name="bo8b")     # +1/8 (k path)
            bo64b = const.tile([P, 65], BF16, name="bo64b")   # +1/64
            bon8b = const.tile([P, 65], BF16, name="bon8b")   # -1/8 (q path)
            for t_, v_ in ((bo8b, 0.125), (bo64b, 1.0 / 64),
                           (bon8b, -0.125)):
                nc.vector.memset(cscr[:], 0.0)
                nc.vector.memset(cscr[0:64, 0:1], v_)
                nc.vector.memset(cscr[64:128, 64:65], v_)
                nc.vector.tensor_copy(t_[:], cscr[:])
            bemb_sb = const.tile([P, O2 // P], F32, name="bemb_sb")
            nc.sync.dma_start(bemb_sb[:], bemb[:])

            pid = nc.gpsimd.partition_id()
            peer = 1 - (pid % 2)

            with tc.tile_pool(name="big", bufs=1) as big:
                k_all = big.tile([P, CH, T], BF16, name="k_all")       # 32KB/part
                v_all = big.tile([P, NMC, H, 72], BF16, name="v_all")  # 36KB
                rk_all = big.tile([P, NMC, H], F32, name="rk_all")     # 1KB
                qa_all = big.tile([P, CH, TQ], BF16, name="qa_all")    # 16KB
                nc.vector.memset(v_all[:, :, :, 64:72], 1.0)

                cc_in = dram.tile([P, CH, CCW], F32, name="cc_in", bufs=1)
                cc_out = dram.tile([2, P, CH, CCW], F32, name="cc_out", bufs=1)
                rk_d = dram.tile([H, TQ], F32, name="rk_d", bufs=1)
                den_d = dram.tile([CH, 2048], F32, name="den_d", bufs=1)
                rec_d = dram.tile([CH, 2048], F32, name="rec_d", bufs=1)

                with tc.tile_pool(name="hp_", bufs=1) as hpool:
                    h_sb = hpool.tile([P, CH, TQ], F32R, name="h_sb")  # 32KB

                    with (
                        tc.tile_pool(name="ps_mm", bufs=3, space="PSUM") as ps_mm,
                        tc.tile_pool(name="ps_row", bufs=2, space="PSUM") as ps_row,
                    ):
                        # ====== A1: se + LN(x) + FiLM -> h (own tokens) ======
                        with (
                            tc.tile_pool(name="a1x", bufs=1) as a1x,
                            tc.tile_pool(name="a1w", bufs=2) as a1w,
                            tc.tile_pool(name="a1s", bufs=2) as a1s,
                            tc.tile_pool(name="a1r", bufs=1) as a1r,
                        ):
                            for tg in range(2):
                                tsl = slice(tg * 512, (tg + 1) * 512)
                                x_t = a1x.tile([P, CH, 512], F32R, name="x_t", tag="x")
                                nc.sync.dma_start(x_t[:], xT_r[:, :, tsl])
                                e_t = a1x.tile([P, CH, 512], F32R, name="e_t", tag="e")
                                nc.sync.dma_start(e_t[:], embT_r[:, :, tsl])

                                # LN stats over channels (partition reduction)
                                ps_s = ps_row.tile([1, 512], F32, name="ps_s", tag="pr")
                                ps_s2 = ps_row.tile([1, 512], F32, name="ps_s2", tag="pr")
                                for ch in range(CH):
                                    x2c = a1s.tile([P, 512], F32R, name="x2c", tag="scr")
                                    nc.gpsimd.tensor_tensor(x2c[:], x_t[:, ch, :],
                                                            x_t[:, ch, :], MUL)
                                    nc.tensor.matmul(ps_s[:], ones_r[:], x_t[:, ch, :],
                                                     start=(ch == 0), stop=(ch == CH - 1))
                                    nc.tensor.matmul(ps_s2[:], ones_r[:], x2c[:],
                                                     start=(ch == 0), stop=(ch == CH - 1))
                                mu = a1r.tile([1, 512], F32, name="mu")
                                vr = a1r.tile([1, 512], F32, name="vr")
                                nmr = a1r.tile([1, 512], F32, name="nmr")
                                nc.vector.tensor_scalar_mul(mu[:], ps_s[:], 1.0 / C)
                                # vr = E[x^2] - mu^2
                                nc.vector.tensor_tensor(vr[:], mu[:], mu[:], MUL)
                                nc.vector.scalar_tensor_tensor(vr[:], ps_s2[:], 1.0 / C,
                                                               vr[:], MUL, SUB)
                                # rstd = exp(-0.5 ln(vr+eps))  (in place)
                                nc.scalar.activation(vr[:], vr[:], LN_,
                                                     bias=eps_t[0:1], scale=1.0)
                                nc.scalar.activation(vr[:], vr[:], EXP,
                                                     bias=0.0, scale=-0.5)
                                nc.vector.scalar_tensor_tensor(nmr[:], mu[:], -1.0,
                                                               vr[:], MUL, MUL)

                                rows_d = dram.tile([2, 512], F32, name="rows_d")
                                nc.sync.dma_start(rows_d[0:1, :], vr[:])
                                nc.sync.dma_start(rows_d[1:2, :], nmr[:])
                                rstd_bc = a1r.tile([P, 512], F32, name="rstd_bc")
                                nc.sync.dma_start(rstd_bc[:],
                                                  rows_d[0:1, :].to_broadcast((P, 512)))
                                nmr_bc = a1r.tile([P, 512], F32, name="nmr_bc")
                                nc.sync.dma_start(nmr_bc[:],
                                                  rows_d[1:2, :].to_broadcast((P, 512)))

                                for ch in range(CH):
                                    wemb_sb = a1w.tile([P, CH, 2, P], F32R,
                                                       name="wemb_sb", tag="wemb")
                                    nc.sync.dma_start(
                                        wemb_sb[:, :, 0, :],
                                        WembT_r[:, :, ch * P:(ch + 1) * P])
                                    nc.sync.dma_start(
                                        wemb_sb[:, :, 1, :],
                                        WembT_r[:, :, C + ch * P:C + (ch + 1) * P])
                                    ps_sc = ps_mm.tile([P, 512], F32, name="ps_sc",
                                                       tag="pm")
                                    ps_sh = ps_mm.tile([P, 512], F32, name="ps_sh",
                                                       tag="pm")
                                    for ech in range(CH):
                                        nc.tensor.matmul(ps_sc[:], wemb_sb[:, ech, 0, :],
                                                         e_t[:, ech, :],
                                                         start=(ech == 0),
                                                         stop=(ech == CH - 1))
                                    for ech in range(CH):
                                        nc.tensor.matmul(ps_sh[:], wemb_sb[:, ech, 1, :],
                                                         e_t[:, ech, :],
                                                         start=(ech == 0),
                                                         stop=(ech == CH - 1))
                                    nc.vector.tensor_scalar_add(ps_sc[:], ps_sc[:],
                                                                bemb_sb[:, ch:ch + 1])
                                    nc.vector.tensor_scalar_add(
                                        ps_sh[:], ps_sh[:],
                                        bemb_sb[:, CH + ch:CH + ch + 1])
                                    xn = a1s.tile([P, 512], F32, name="xn", tag="scr2")
                                    nc.gpsimd.tensor_tensor(xn[:], x_t[:, ch, :],
                                                            rstd_bc[:], MUL)
                                    nc.gpsimd.tensor_tensor(xn[:], xn[:], nmr_bc[:], ADD)
                                    nc.vector.tensor_tensor(xn[:], xn[:], ps_sc[:], MUL)
                                    nc.vector.tensor_tensor(h_sb[:, ch, tsl], xn[:],
                                                            ps_sh[:], ADD)

                        # ====== A2: own k/v/rk + pair exchange ======
                        with (
                            tc.tile_pool(name="a2w", bufs=2) as a2w,
                            tc.tile_pool(name="a2r", bufs=2) as a2r,
                            tc.tile_pool(name="a2s", bufs=2) as a2s,
                        ):
                            for hp in range(CH):
                                wk_sb = a2w.tile([P, CH, P], F32R, name="wk_sb",
                                                 tag="wk")
                                nc.sync.dma_start(
                                    wk_sb[:],
                                    WprojT_r[:, :, C + hp * P:C + (hp + 1) * P])
                                vark = a2s.tile([65, TQ], F32, name="vark", tag="vark")
                                for mt in range(NMT):
                                    msl = slice(mt * 512, (mt + 1) * 512)
                                    ps_k = ps_mm.tile([P, 512], F32, name="ps_k",
                                                      tag="pm")
                                    for ch in range(CH):
                                        nc.tensor.matmul(ps_k[:], wk_sb[:, ch, :],
                                                         h_sb[:, ch, msl],
                                                         start=(ch == 0),
                                                         stop=(ch == CH - 1))
                                    nc.vector.tensor_copy(k_all[:, hp, msl], ps_k[:])
                                    ksq = a2r.tile([P, 512], BF16, name="ksq",
                                                   tag="ksq")
                                    nc.gpsimd.tensor_tensor(ksq[:], k_all[:, hp, msl],
                                                            k_all[:, hp, msl], MUL)
                                    ps_kr = ps_row.tile([65, 512], F32, name="ps_kr",
                                                        tag="pr")
                                    nc.tensor.matmul(ps_kr[:], bo8b[:],
                                                     k_all[:, hp, msl],
                                                     start=True, stop=True)
                                    ps_kr2 = ps_row.tile([65, 512], F32, name="ps_kr2",
                                                         tag="pr")
                                    nc.tensor.matmul(ps_kr2[:], bo64b[:], ksq[:],
                                                     start=True, stop=True)
                                    # vark = E[k^2] - mu^2
                                    m2k = a2r.tile([65, 512], F32, name="m2k",
                                                   tag="m2k")
                                    nc.vector.tensor_scalar_mul(m2k[:], ps_kr[:],
                                                                0.125)
                                    nc.vector.tensor_tensor(m2k[:], m2k[:], m2k[:],
                                                            MUL)
                                    nc.vector.scalar_tensor_tensor(vark[:, msl],
                                                                   ps_kr2[:], 1.0,
                                                                   m2k[:], MUL, SUB)
                                # rk/8 = exp(-0.5 ln(vark+eps) - ln8)
                                nc.scalar.activation(vark[:], vark[:], LN_,
                                                     bias=eps_t[0:65], scale=1.0)
                                nc.scalar.activation(vark[:], vark[:], EXP,
                                                     bias=nl8_t[0:65], scale=-0.5)
                                nc.sync.dma_start(rk_d[2 * hp:2 * hp + 1, :],
                                                  vark[0:1, :])
                                nc.sync.dma_start(rk_d[2 * hp + 1:2 * hp + 2, :],
                                                  vark[64:65, :])

                            # v projection: token-major, 8 heads per matmul
                            for vh in range(2):
                                wv_sb = a2w.tile([P, CH, 512], F32R, name="wv_sb",
                                                 tag="wv")
                                nc.sync.dma_start(
                                    wv_sb[:],
                                    WprojT_r[:, :,
                                             2 * C + vh * 512:2 * C + (vh + 1) * 512])
                                for mc in range(NMCH):
                                    ps_v = ps_mm.tile([P, 512], F32, name="ps_v",
                                                      tag="pm")
                                    for ch in range(CH):
                                        nc.tensor.matmul(
                                            ps_v[:], h_sb[:, ch, mc * P:(mc + 1) * P],
                                            wv_sb[:, ch, :],
                                            start=(ch == 0), stop=(ch == CH - 1))
                                    nc.vector.tensor_copy(
                                        v_all[:, mc, vh * 8:(vh + 1) * 8, 0:64],
                                        ps_v[:])

                            # rk column form (own half) straight from DRAM rows
                            for mc in range(NMCH):
                                nc.sync.dma_start(
                                    rk_all[:, mc, :],
                                    rk_d[:, mc * P:(mc + 1) * P].rearrange(
                                        "h p -> p h"))

                            # pack own payload + pair AllGather + peer readback
                            nc.gpsimd.dma_start(cc_in[:, :, 0:512],
                                                k_all[:, :, 0:TQ].bitcast(F32))
                            nc.gpsimd.dma_start(
                                cc_in[:, :, 512:1088],
                                v_all[:, 0:NMCH, :, :].bitcast(F32).rearrange(
                                    "p a b c -> p a (b c)"))
                            nc.gpsimd.dma_start(cc_in[:, :, 1088:CCW],
                                                rk_all[:, 0:NMCH, :])
                            nc.gpsimd.collective_compute(
                                "AllGather",
                                mybir.AluOpType.bypass,
                                replica_groups=[[0, 1], [2, 3], [4, 5], [6, 7]],
                                ins=[cc_in[:]],
                                outs=[cc_out[:]],
                            )
                            nc.gpsimd.dma_start(k_all[:, :, TQ:T].bitcast(F32),
                                                cc_out[peer, :, :, 0:512])
                            nc.gpsimd.dma_start(
                                v_all[:, NMCH:NMC, :, :].bitcast(F32).rearrange(
                                    "p a b c -> p a (b c)"),
                                cc_out[peer, :, :, 512:1088])
                            nc.gpsimd.dma_start(rk_all[:, NMCH:NMC, :],
                                                cc_out[peer, :, :, 1088:CCW])

                        # ====== Q: q-proj + centered/scaled qa, all hp ======
                        with (
                            tc.tile_pool(name="qw", bufs=2) as qw,
                            tc.tile_pool(name="qr", bufs=2) as qr,
                        ):
                            for hp in range(CH):
                                wq_sb = qw.tile([P, CH, P], F32R, name="wq_sb",
                                                tag="wq")
                                nc.sync.dma_start(
                                    wq_sb[:], WprojT_r[:, :, hp * P:(hp + 1) * P])
                                nsq8 = qr.tile([65, TQ], F32, name="nsq8", tag="nsq8")
                                ex2q = qr.tile([65, TQ], F32, name="ex2q", tag="ex2q")
                                for nt in range(NNT):
                                    nsl = slice(nt * 512, (nt + 1) * 512)
                                    ps_q = ps_mm.tile([P, 512], F32, name="ps_q",
                                                      tag="pm")
                                    for ch in range(CH):
                                        nc.tensor.matmul(ps_q[:], wq_sb[:, ch, :],
                                                         h_sb[:, ch, nsl],
                                                         start=(ch == 0),
                                                         stop=(ch == CH - 1))
                                    nc.vector.tensor_copy(qa_all[:, hp, nsl], ps_q[:])
                                    qsq = qr.tile([P, 512], BF16, name="qsq", tag="qsq")
                                    nc.gpsimd.tensor_tensor(qsq[:], qa_all[:, hp, nsl],
                                                            qa_all[:, hp, nsl], MUL)
                                    ps_qr = ps_row.tile([65, 512], F32, name="ps_qr",
                                                        tag="pr")
                                    nc.tensor.matmul(ps_qr[:], bon8b[:],
                                                     qa_all[:, hp, nsl],
                                                     start=True, stop=True)
                                    ps_qr2 = ps_row.tile([65, 512], F32, name="ps_qr2",
                                                         tag="pr")
                                    nc.tensor.matmul(ps_qr2[:], bo64b[:], qsq[:],
                                                     start=True, stop=True)
                                    nc.vector.tensor_copy(nsq8[:, nsl], ps_qr[:])
                                    nc.vector.tensor_copy(ex2q[:, nsl], ps_qr2[:])
                                # rq = exp(-0.5 ln(var+eps)); numr = mu*rq
                                rq = qr.tile([65, TQ], F32, name="rq", tag="rq")
                                nc.vector.scalar_tensor_tensor(rq[:], nsq8[:], 1.0 / 64,
                                                               nsq8[:], MUL, MUL)
                                nc.vector.tensor_tensor(rq[:], ex2q[:], rq[:], SUB)
                                nc.scalar.activation(rq[:], rq[:], LN_,
                                                     bias=eps_t[0:65], scale=1.0)
                                nc.scalar.activation(rq[:], rq[:], EXP,
                                                     bias=0.0, scale=-0.5)
                                numr = qr.tile([65, TQ], F32, name="numr", tag="numr")
                                nc.vector.scalar_tensor_tensor(numr[:], nsq8[:], -0.125,
                                                               rq[:], MUL, MUL)
                                rowq_d = dram.tile([4, TQ], F32, name="rowq_d")
                                nc.sync.dma_start(rowq_d[0:1, :], rq[0:1, :])
                                nc.sync.dma_start(rowq_d[1:2, :], rq[64:65, :])
                                nc.sync.dma_start(rowq_d[2:3, :], numr[0:1, :])
                                nc.sync.dma_start(rowq_d[3:4, :], numr[64:65, :])
                                rq_bc = qr.tile([P, TQ], F32, name="rq_bc", tag="rqbc")
                                nc.sync.dma_start(rq_bc[0:64, :],
                                                  rowq_d[0:1, :].to_broadcast((64, TQ)))
                                nc.sync.dma_start(rq_bc[64:128, :],
                                                  rowq_d[1:2, :].to_broadcast((64, TQ)))
                                nm_bc = qr.tile([P, TQ], F32, name="nm_bc", tag="nmbc")
                                nc.sync.dma_start(nm_bc[0:64, :],
                                                  rowq_d[2:3, :].to_broadcast((64, TQ)))
                                nc.sync.dma_start(nm_bc[64:128, :],
                                                  rowq_d[3:4, :].to_broadcast((64, TQ)))
                                # qa = q*rq - mu*rq   (in place over raw q)
                                nc.vector.tensor_tensor(qa_all[:, hp, :],
                                                        qa_all[:, hp, :], rq_bc[:], MUL)
                                nc.gpsimd.tensor_tensor(qa_all[:, hp, :],
                                                        qa_all[:, hp, :], nm_bc[:], SUB)

                # h + A-phase PSUM pools freed here
                with (
                    tc.tile_pool(name="ofm", bufs=1) as ofm,
                ):
                    o_fm = ofm.tile([P, CH, TQ], F32R, name="o_fm")

                    with (
                        tc.tile_pool(name="psb", bufs=3) as psb,
                        tc.tile_pool(name="dsbp", bufs=2) as dsbp,
                        tc.tile_pool(name="recb", bufs=2) as recb,
                        tc.tile_pool(name="ps_att", bufs=2, space="PSUM") as ps_att,
                        tc.tile_pool(name="ps_av", bufs=1, space="PSUM") as ps_av,
                    ):
                        def att_pass(hp, mcs):
                            """scores+exp+av over key-chunks mcs for head-pair hp."""
                            av = ps_av.tile([65, 2, NNT, 512], F32, name="av",
                                            tag="av")
                            for i, mc in enumerate(mcs):
                                first, last = i == 0, i == len(mcs) - 1
                                for hh in range(2):
                                    g = 2 * hp + hh
                                    b0 = hh * 64
                                    sc = ps_att.tile([P, NNT, 512], F32, name="sc",
                                                     tag="sc")
                                    for nt in range(NNT):
                                        nc.tensor.matmul(
                                            sc[:, nt, :],
                                            k_all[b0:b0 + 64, hp, mc * P:(mc + 1) * P],
                                            qa_all[b0:b0 + 64, hp,
                                                   nt * 512:(nt + 1) * 512],
                                            start=True, stop=True)
                                    p_sb = psb.tile([P, NNT, 512], BF16, name="p_sb",
                                                    tag="p")
                                    nc.scalar.activation(p_sb[:], sc[:], EXP, bias=0.0,
                                                         scale=rk_all[:, mc, g:g + 1])
                                    for nt in range(NNT):
                                        nc.tensor.matmul(av[:, hh, nt, :],
                                                         v_all[:, mc, g, 0:65],
                                                         p_sb[:, nt, :],
                                                         start=first, stop=last)
                            return av

                        # ---- pass 1: own key chunks (overlaps the exchange) ----
                        for hp in range(CH):
                            av = att_pass(hp, range(NMCH))
                            dsb = dsbp.tile([65, 2048], F32, name="dsb", tag="dsb")
                            nc.vector.tensor_copy(
                                dsb[64:65, :],
                                av[64:65, :, :, :].rearrange("a b c d -> a (b c d)"))
                            nc.gpsimd.dma_start(den_d[hp:hp + 1, :], dsb[64:65, :])
                            for hh in range(2):
                                g = 2 * hp + hh
                                nc.vector.tensor_copy(
                                    o_fm[(g % 2) * 64:(g % 2) * 64 + 64, g // 2, :],
                                    av[0:64, hh, :, :].rearrange("a b c -> a (b c)"))

                        # ---- pass 2: peer key chunks + combine/divide ----
                        for hp in range(CH):
                            av = att_pass(hp, range(NMCH, NMC))
                            dsb = dsbp.tile([65, 2048], F32, name="dsb", tag="dsb")
                            nc.gpsimd.dma_start(dsb[64:65, :], den_d[hp:hp + 1, :])
                            nc.vector.tensor_tensor(
                                dsb[64:65, :], dsb[64:65, :],
                                av[64:65, :, :, :].rearrange("a b c d -> a (b c d)"),
                                ADD)
                            # rec = exp(-ln(den))
                            nc.scalar.activation(dsb[64:65, :], dsb[64:65, :], LN_,
                                                 bias=eps_t[64:65], scale=1.0)
                            nc.scalar.activation(dsb[64:65, :], dsb[64:65, :], EXP,
                                                 bias=0.0, scale=-1.0)
                            nc.gpsimd.dma_start(rec_d[hp:hp + 1, :], dsb[64:65, :])
                            rec_bc = recb.tile([P, TQ], F32, name="rec_bc", tag="rec")
                            nc.sync.dma_start(
                                rec_bc[0:64, :],
                                rec_d[hp:hp + 1, 0:TQ].to_broadcast((64, TQ)))
                            nc.sync.dma_start(
                                rec_bc[64:128, :],
                                rec_d[hp:hp + 1, TQ:2048].to_broadcast((64, TQ)))
                            for hh in range(2):
                                g = 2 * hp + hh
                                osl = o_fm[(g % 2) * 64:(g % 2) * 64 + 64, g // 2, :]
                                nc.vector.tensor_tensor(
                                    osl, osl,
                                    av[0:64, hh, :, :].rearrange("a b c -> a (b c)"),
                                    ADD)
                                nc.gpsimd.tensor_tensor(
                                    osl, osl, rec_bc[hh * 64:(hh + 1) * 64, :], MUL)

                    # ====== C: out = o_fm.T @ (I + W_out).T ======
                    with (
                        tc.tile_pool(name="cw", bufs=2) as cw,
                        tc.tile_pool(name="ps_c", bufs=3, space="PSUM") as ps_c,
                    ):
                        for jt in range(C // 512):
                            wres_sb = cw.tile([P, CH, 512], F32R, name="wres_sb",
                                              tag="wres")
                            nc.sync.dma_start(
                                wres_sb[:],
                                WresT_r[:, :, jt * 512:(jt + 1) * 512])
                            for ns in range(TQ // P):
                                ps_f = ps_c.tile([P, 512], F32, name="ps_f", tag="pc")
                                for cg in range(CH):
                                    nc.tensor.matmul(
                                        ps_f[:], o_fm[:, cg, ns * P:(ns + 1) * P],
                                        wres_sb[:, cg, :],
                                        start=(cg == 0), stop=(cg == CH - 1))
                                f_sb = cw.tile([P, 512], F32, name="f_sb", tag="fsb")
                                nc.vector.tensor_copy(f_sb[:], ps_f[:])
                                nc.gpsimd.dma_start(
                                    out[ns * P:(ns + 1) * P,
                                        jt * 512:(jt + 1) * 512],
                                    f_sb[:])

    nc.finalize()
    return nc


def _prep_host(x, emb, W_emb, b_emb, W_proj, W_out):
    W_embT = np.ascontiguousarray(W_emb.T.astype(np.float32))
    W_projT = np.ascontiguousarray(W_proj.T.astype(np.float32))
    W_resT = np.ascontiguousarray(
        (np.eye(C, dtype=np.float32) + W_out).T.astype(np.float32))
    bemb2 = b_emb.astype(np.float32).copy()
    bemb2[:C] += 1.0                       # fold the FiLM "+1" into the bias
    bemb_col = np.ascontiguousarray(bemb2.reshape(O2 // P, P).T)

    in_maps = []
    for c in range(NCORES):
        b, j = c // 2, c % 2
        tsl = slice(j * TQ, (j + 1) * TQ)
        in_maps.append({
            "xT": np.ascontiguousarray(x[b, tsl].T.astype(np.float32)),
            "embT": np.ascontiguousarray(emb[b, tsl].T.astype(np.float32)),
            "WembT": W_embT, "bemb": bemb_col,
            "WprojT": W_projT, "WresT": W_resT,
        })
    return in_maps


def kernel(x, emb, W_emb, b_emb, W_proj, W_out, _trace=False):
    x = np.asarray(x); emb = np.asarray(emb)
    W_emb = np.asarray(W_emb); b_emb = np.asarray(b_emb)
    W_proj = np.asarray(W_proj); W_out = np.asarray(W_out)

    if "nc" not in _cached:
        _cached["nc"] = build_kernel()
    nc = _cached["nc"]

    in_maps = _prep_host(x, emb, W_emb, b_emb, W_proj, W_out)
    res = run_bass_kernel_spmd(nc, in_maps, core_ids=list(range(NCORES)),
                               trace=_trace)
    _cached["last_result"] = res

    outp = np.empty((B, N, C), dtype=np.float32)
    for c in range(NCORES):
        b, j = c // 2, c % 2
        outp[b, j * TQ:(j + 1) * TQ, :] = res.results[c]["out"]
    return outp


if __name__ == "__main__":
    build_kernel()
    print("BUILD OK")


# revision 14
# speedup vs baseline: 1.5712x; 1.1283x over previous
"""AttentionBlock Trainium2 kernel (B=4, N=2048, C=1024, H=16, D=64, EMB=1024).

    se = emb @ W_emb.T + b_emb;  scale, shift = split(se, 2, -1)
    h  = LN(x) * (1+scale) + shift
    q,k,v = split(h @ W_proj.T) -> (B,H,N,D);  q = LN(q); k = LN(k)  (over D)
    o  = softmax(q k^T / sqrt(D)) v  -> (B,N,C)
    out = o + o @ W_out.T

Sharding: 8 cores; core c -> batch b=c//2, token-half j=c%2 (natural order).
Each core computes the preamble (se/h/k/v/stats) only for its OWN 1024
tokens, then token-pair cores exchange k/v/rk via a pair-wise AllGather
(DRAM bounce); the peer block is read back with a register-offset DMA
(peer = 1 - pid%2), keeping one uniform SPMD NEFF. Phase order puts the
exchange as early as possible (A1 -> v-proj -> k-proj -> pack+AllGather),
so the collective overlaps the q phase and pass1 of attention (own
key-chunks); pass2 (peer chunks) combines the softmax partials at the end.

Dataflow is feature-major (channels on partitions) end to end:
  - LayerNorm over channels == partition reduction -> ones-column matmuls.
  - Per-token (free-dim) row scalars are broadcast across partitions with a
    K=1 ones-row matmul into PSUM (no DRAM round-trip, no sync-queue chain).
  - q LN-centering folds into q (qa = q*rq - mu_q*rq); k needs NO centering
    because sum_d qa[d] = 0 kills the mu_k cross term exactly.
  - k LN-scaling rides the ACT exp as a per-partition scale operand (rk/8).
  - Scores use 64-row 2-head tile_position row-tiling (2 heads concurrent
    in the PE array); exp is batched [128, 2x512] over both query tiles.
  - Softmax denominators come free as a ones-column appended to v; the
    division happens once at the end of pass2 as a gpsimd tensor divide.
  - rsqrt via exp(-0.5 ln x) on ACT; an explicit ACT_TABLE_LOAD of the
    ln+exp set is emitted up front so the whole kernel uses ONE table set.
  - The residual is folded into the output projection: W_res = (I+W_out).T.

Matmuls in float32r (1 cycle/row); q/k/p/v in bf16 (errors largely cancel
in the softmax ratio).
"""

import sys

sys.path.insert(0, "/opt/trn_rl_repo")

import numpy as np

import concourse.bass as bass
import concourse.mybir as mybir
import concourse.tile as tile
from concourse import bacc
from concourse.bass_utils import run_bass_kernel_spmd

P = 128
B, N, C = 4, 2048, 1024
H, D = 16, 64
EMB = 1024
EPS = 1e-5
T = N          # tokens per batch (k/v coverage after exchange)
TQ = N // 2    # own tokens per core
CH = C // P    # 8 channel chunks
O2 = 2 * C
NCORES = 8
NMC = T // P       # 16 key-token 128-chunks (8 own + 8 peer)
NMCH = NMC // 2    # 8
NNT = TQ // 512    # 2 query-token 512-tiles
NMT = TQ // 512    # 2 own 512-tiles for projections

# packed pair-exchange payload, f32 units, 8 slots per partition:
#   [0:512]     k  (bf16 x1024 viewed as f32 x512)    slot = head-pair hp
#   [512:1088]  v  (bf16 [16 heads, 72] as f32 x576)  slot = own key-chunk mc
#   [1088:1104] rk/8 column form [16 heads]           slot = own key-chunk mc
CCW = 1104

F32 = mybir.dt.float32
F32R = mybir.dt.float32r
BF16 = mybir.dt.bfloat16
MUL = mybir.AluOpType.mult
ADD = mybir.AluOpType.add
SUB = mybir.AluOpType.subtract
DIV = mybir.AluOpType.divide
EXP = mybir.ActivationFunctionType.Exp
LN_ = mybir.ActivationFunctionType.Ln
NL8 = -2.0794415416798357  # -ln(8)

_cached = {}


def _lnexp_set_id(nc):
    """Index of the activation-table set containing both Exp and Ln."""
    try:
        from concourse.hw_specs import get_activation_tables
        tabs = list(get_activation_tables(nc.m.arch).items())
        return next(i for i, (_, fns) in enumerate(tabs)
                    if EXP in fns and LN_ in fns)
    except Exception:
        return 6  # natural_log_exp_and_others in the shipped act_info.json


def build_kernel():
    nc = bacc.Bacc()

    xT = nc.dram_tensor("xT", [C, TQ], F32R, kind="ExternalInput")
    embT = nc.dram_tensor("embT", [EMB, TQ], F32R, kind="ExternalInput")
    WembT = nc.dram_tensor("WembT", [EMB, O2], F32R, kind="ExternalInput")
    bemb = nc.dram_tensor("bemb", [P, O2 // P], F32, kind="ExternalInput")
    WprojT = nc.dram_tensor("WprojT", [C, 3 * C], F32R, kind="ExternalInput")
    WresT = nc.dram_tensor("WresT", [C, C], F32R, kind="ExternalInput")
    out = nc.dram_tensor("out", [TQ, C], F32, kind="ExternalOutput")

    xT_r = xT.rearrange("(ch p) t -> p ch t", p=P)
    embT_r = embT.rearrange("(ch p) t -> p ch t", p=P)
    WembT_r = WembT.rearrange("(ch p) o -> p ch o", p=P)
    WprojT_r = WprojT.rearrange("(ch p) o -> p ch o", p=P)
    WresT_r = WresT.rearrange("(ch p) o -> p ch o", p=P)

    with tile.TileContext(nc) as tc:
        with (
            tc.tile_pool(name="const", bufs=1) as const,
            tc.tile_pool(name="dram", bufs=2, space="DRAM") as dram,
        ):
            # one table set (ln+exp) for the entire kernel
            nc.scalar.add_instruction(mybir.InstLoadActFuncSet(
                name=nc.get_next_instruction_name(), ins=[], outs=[],
                act_func_set_id=_lnexp_set_id(nc)))

            # ---------------- constants ----------------
            eps_t = const.tile([P, 1], F32, name="eps_t")
            nc.vector.memset(eps_t[:], EPS)
            nl8_t = const.tile([P, 1], F32, name="nl8_t")
            nc.vector.memset(nl8_t[:], NL8)
            cscr = const.tile([P, 128], F32, name="cscr")
            ones_r = const.tile([P, 1], F32R, name="ones_r")
            nc.vector.memset(cscr[:, 0:1], 1.0)
            nc.vector.tensor_copy(ones_r[:], cscr[:, 0:1])
            # ones ROW for K=1 broadcast matmuls (partitions 0 and 64)
            onesrow = const.tile([65, 128], F32R, name="onesrow")
            nc.vector.memset(cscr[:], 1.0)
            nc.vector.tensor_copy(onesrow[:], cscr[0:65, :])
            # 2-row broadcast selector: out[p] = row0 for p<64, row64 for p>=64
            bsel = const.tile([65, 128], F32R, name="bsel")
            nc.vector.memset(cscr[:], 0.0)
            nc.vector.memset(cscr[0:1, 0:64], 1.0)
            nc.vector.memset(cscr[64:65, 64:128], 1.0)
            nc.vector.tensor_copy(bsel[:], cscr[0:65, :])
            # per-head partition-block sums: lhsT cols 0 and 64 select heads.
            bo8b = const.tile([P, 65], BF16, name="bo8b")     # +1/8 (k path)
            bo64b = const.tile([P, 65], BF16, name="bo64b")   # +1/64
            bon8b = const.tile([P, 65], BF16, name="bon8b")   # -1/8 (q path)
            for t_, v_ in ((bo8b, 0.125), (bo64b, 1.0 / 64), (bon8b, -0.125)):
                nc.vector.memset(cscr[:], 0.0)
                nc.vector.memset(cscr[0:64, 0:1], v_)
                nc.vector.memset(cscr[64:128, 64:65], v_)
                nc.vector.tensor_copy(t_[:], cscr[:, 0:65])
            bemb_sb = const.tile([P, O2 // P], F32, name="bemb_sb")
            nc.sync.dma_start(bemb_sb[:], bemb[:])

            peer_sp = 1 - (nc.sync.partition_id() % 2)

            with tc.tile_pool(name="big", bufs=1) as big:
                k_all = big.tile([P, CH, T], BF16, name="k_all")       # 32KB/part
                v_all = big.tile([P, NMC, H, 72], BF16, name="v_all")  # 36KB
                rk_all = big.tile([P, NMC, H], F32, name="rk_all")     # 1KB
                qa_all = big.tile([P, CH, TQ], BF16, name="qa_all")    # 16KB
                nc.vector.memset(v_all[:, :, :, 64:72], 1.0)

                cc_in = dram.tile([P, CH, CCW], F32, name="cc_in", bufs=1)
                cc_out = dram.tile([2, P, CH, CCW], F32, name="cc_out", bufs=1)
                rk_d = dram.tile([H, TQ], F32, name="rk_d", bufs=1)
                den_d = dram.tile([CH, 2048], F32, name="den_d", bufs=1)

                with tc.tile_pool(name="hp_", bufs=1) as hpool:
                    h_sb = hpool.tile([P, CH, TQ], F32R, name="h_sb")  # 32KB

                    with (
                        tc.tile_pool(name="ps_mm", bufs=3, space="PSUM") as ps_mm,
                        tc.tile_pool(name="ps_row", bufs=2, space="PSUM") as ps_row,
                        tc.tile_pool(name="ps_bc", bufs=2, space="PSUM") as ps_bc,
                    ):
                        # ====== A1: se + LN(x) + FiLM -> h (own tokens) ======
                        with (
                            tc.tile_pool(name="a1x", bufs=1) as a1x,
                            tc.tile_pool(name="a1w", bufs=2) as a1w,
                            tc.tile_pool(name="a1s", bufs=2) as a1s,
                            tc.tile_pool(name="a1r", bufs=1) as a1r,
                        ):
                            for tg in range(2):
                                tsl = slice(tg * 512, (tg + 1) * 512)
                                x_t = a1x.tile([P, CH, 512], F32R, name="x_t", tag="x")
                                nc.sync.dma_start(x_t[:], xT_r[:, :, tsl])
                                e_t = a1x.tile([P, CH, 512], F32R, name="e_t", tag="e")
                                nc.sync.dma_start(e_t[:], embT_r[:, :, tsl])

                                # LN stats over channels (partition reduction)
                                ps_s = ps_row.tile([1, 512], F32, name="ps_s", tag="pr")
                                ps_s2 = ps_row.tile([1, 512], F32, name="ps_s2", tag="pr")
                                for ch in range(CH):
                                    x2c = a1s.tile([P, 512], F32R, name="x2c", tag="scr")
                                    nc.gpsimd.tensor_tensor(x2c[:], x_t[:, ch, :],
                                                            x_t[:, ch, :], MUL)
                                    nc.tensor.matmul(ps_s[:], ones_r[:], x_t[:, ch, :],
                                                     start=(ch == 0), stop=(ch == CH - 1))
                                    nc.tensor.matmul(ps_s2[:], ones_r[:], x2c[:],
                                                     start=(ch == 0), stop=(ch == CH - 1))
                                mu = a1r.tile([1, 512], F32, name="mu")
                                vr = a1r.tile([1, 512], F32R, name="vr")
                                nmr = a1r.tile([1, 512], F32R, name="nmr")
                                nc.vector.tensor_scalar_mul(mu[:], ps_s[:], 1.0 / C)
                                # vr = E[x^2] - mu^2
                                nc.vector.tensor_tensor(vr[:], mu[:], mu[:], MUL)
                                nc.vector.scalar_tensor_tensor(vr[:], ps_s2[:], 1.0 / C,
                                                               vr[:], MUL, SUB)
                                # rstd = exp(-0.5 ln(vr+eps))  (in place)
                                nc.scalar.activation(vr[:], vr[:], LN_,
                                                     bias=eps_t[0:1], scale=1.0)
                                nc.scalar.activation(vr[:], vr[:], EXP,
                                                     bias=0.0, scale=-0.5)
                                nc.vector.scalar_tensor_tensor(nmr[:], mu[:], -1.0,
                                                               vr[:], MUL, MUL)

                                # broadcast rows across partitions via K=1 matmul
                                bc_r = ps_bc.tile([P, 512], F32, name="bc_r", tag="bc")
                                nc.tensor.matmul(bc_r[:], onesrow[0:1, :], vr[:],
                                                 start=True, stop=True)
                                bc_n = ps_bc.tile([P, 512], F32, name="bc_n", tag="bc")
                                nc.tensor.matmul(bc_n[:], onesrow[0:1, :], nmr[:],
                                                 start=True, stop=True)
                                rstd_sb = a1r.tile([P, 512], F32, name="rstd_sb")
                                nc.vector.tensor_copy(rstd_sb[:], bc_r[:])
                                nmr_sb = a1r.tile([P, 512], F32, name="nmr_sb")
                                nc.vector.tensor_copy(nmr_sb[:], bc_n[:])

                                for ch in range(CH):
                                    wemb_sb = a1w.tile([P, CH, 2, P], F32R,
                                                       name="wemb_sb", tag="wemb")
                                    nc.sync.dma_start(
                                        wemb_sb[:, :, 0, :],
                                        WembT_r[:, :, ch * P:(ch + 1) * P])
                                    nc.sync.dma_start(
                                        wemb_sb[:, :, 1, :],
                                        WembT_r[:, :, C + ch * P:C + (ch + 1) * P])
                                    ps_sc = ps_mm.tile([P, 512], F32, name="ps_sc",
                                                       tag="pm")
                                    ps_sh = ps_mm.tile([P, 512], F32, name="ps_sh",
                                                       tag="pm")
                                    for ech in range(CH):
                                        nc.tensor.matmul(ps_sc[:], wemb_sb[:, ech, 0, :],
                                                         e_t[:, ech, :],
                                                         start=(ech == 0),
                                                         stop=(ech == CH - 1))
                                    for ech in range(CH):
                                        nc.tensor.matmul(ps_sh[:], wemb_sb[:, ech, 1, :],
                                                         e_t[:, ech, :],
                                                         start=(ech == 0),
                                                         stop=(ech == CH - 1))
                                    nc.vector.tensor_scalar_add(ps_sc[:], ps_sc[:],
                                                                bemb_sb[:, ch:ch + 1])
                                    nc.vector.tensor_scalar_add(
                                        ps_sh[:], ps_sh[:],
                                        bemb_sb[:, CH + ch:CH + ch + 1])
                                    xn = a1s.tile([P, 512], F32, name="xn", tag="scr2")
                                    nc.gpsimd.tensor_tensor(xn[:], x_t[:, ch, :],
                                                            rstd_sb[:], MUL)
                                    nc.gpsimd.tensor_tensor(xn[:], xn[:], nmr_sb[:], ADD)
                                    nc.vector.tensor_tensor(xn[:], xn[:], ps_sc[:], MUL)
                                    nc.vector.tensor_tensor(h_sb[:, ch, tsl], xn[:],
                                                            ps_sh[:], ADD)

                        # ====== A2: own v, k, rk; pack + AllGather asap ======
                        with (
                            tc.tile_pool(name="a2w", bufs=2) as a2w,
                            tc.tile_pool(name="a2r", bufs=2) as a2r,
                            tc.tile_pool(name="a2s", bufs=2) as a2s,
                        ):
                            # v projection: token-major, 8 heads per matmul
                            for vh in range(2):
                                wv_sb = a2w.tile([P, CH, 512], F32R, name="wv_sb",
                                                 tag="wv")
                                nc.sync.dma_start(
                                    wv_sb[:],
                                    WprojT_r[:, :,
                                             2 * C + vh * 512:2 * C + (vh + 1) * 512])
                                for mc in range(NMCH):
                                    ps_v = ps_mm.tile([P, 512], F32, name="ps_v",
                                                      tag="pm")
                                    for ch in range(CH):
                                        nc.tensor.matmul(
                                            ps_v[:], h_sb[:, ch, mc * P:(mc + 1) * P],
                                            wv_sb[:, ch, :],
                                            start=(ch == 0), stop=(ch == CH - 1))
                                    nc.vector.tensor_copy(
                                        v_all[:, mc, vh * 8:(vh + 1) * 8, 0:64],
                                        ps_v[:])
                            nc.gpsimd.dma_start(
                                cc_in[:, :, 512:1088],
                                v_all[:, 0:NMCH, :, :].bitcast(F32).rearrange(
                                    "p a b c -> p a (b c)"))

                            # k projection + stats
                            for hp in range(CH):
                                wk_sb = a2w.tile([P, CH, P], F32R, name="wk_sb",
                                                 tag="wk")
                                nc.sync.dma_start(
                                    wk_sb[:],
                                    WprojT_r[:, :, C + hp * P:C + (hp + 1) * P])
                                vark = a2s.tile([65, TQ], F32, name="vark", tag="vark")
                                for mt in range(NMT):
                                    msl = slice(mt * 512, (mt + 1) * 512)
                                    ps_k = ps_mm.tile([P, 512], F32, name="ps_k",
                                                      tag="pm")
                                    for ch in range(CH):
                                        nc.tensor.matmul(ps_k[:], wk_sb[:, ch, :],
                                                         h_sb[:, ch, msl],
                                                         start=(ch == 0),
                                                         stop=(ch == CH - 1))
                                    nc.vector.tensor_copy(k_all[:, hp, msl], ps_k[:])
                                    ksq = a2r.tile([P, 512], BF16, name="ksq",
                                                   tag="ksq")
                                    nc.gpsimd.tensor_tensor(ksq[:], k_all[:, hp, msl],
                                                            k_all[:, hp, msl], MUL)
                                    ps_kr = ps_row.tile([65, 512], F32, name="ps_kr",
                                                        tag="pr")
                                    nc.tensor.matmul(ps_kr[:], bo8b[:],
                                                     k_all[:, hp, msl],
                                                     start=True, stop=True)
                                    ps_kr2 = ps_row.tile([65, 512], F32, name="ps_kr2",
                                                         tag="pr")
                                    nc.tensor.matmul(ps_kr2[:], bo64b[:], ksq[:],
                                                     start=True, stop=True)
                                    # vark = E[k^2] - mu^2
                                    m2k = a2r.tile([65, 512], F32, name="m2k",
                                                   tag="m2k")
                                    nc.vector.tensor_scalar_mul(m2k[:], ps_kr[:],
                                                                0.125)
                                    nc.vector.tensor_tensor(m2k[:], m2k[:], m2k[:],
                                                            MUL)
                                    nc.vector.scalar_tensor_tensor(vark[:, msl],
                                                                   ps_kr2[:], 1.0,
                                                                   m2k[:], MUL, SUB)
                                # rk/8 = exp(-0.5 ln(vark+eps) - ln8)
                                nc.scalar.activation(vark[:], vark[:], LN_,
                                                     bias=eps_t[0:65], scale=1.0)
                                nc.scalar.activation(vark[:], vark[:], EXP,
                                                     bias=nl8_t[0:65], scale=-0.5)
                                nc.gpsimd.dma_start(rk_d[2 * hp:2 * hp + 1, :],
                                                    vark[0:1, :])
                                nc.gpsimd.dma_start(rk_d[2 * hp + 1:2 * hp + 2, :],
                                                    vark[64:65, :])
                            nc.gpsimd.dma_start(cc_in[:, :, 0:512],
                                                k_all[:, :, 0:TQ].bitcast(F32))
                            # rk column form (own half) from the DRAM rows
                            for mc in range(NMCH):
                                nc.gpsimd.dma_start(
                                    rk_all[:, mc, :],
                                    rk_d[:, mc * P:(mc + 1) * P].rearrange(
                                        "h p -> p h"))
                            nc.gpsimd.dma_start(cc_in[:, :, 1088:CCW],
                                                rk_all[:, 0:NMCH, :])
                            nc.gpsimd.collective_compute(
                                "AllGather",
                                mybir.AluOpType.bypass,
                                replica_groups=[[0, 1], [2, 3], [4, 5], [6, 7]],
                                ins=[cc_in[:]],
                                outs=[cc_out[:]],
                            )
                            # peer readback on the SYNC queue so the gpsimd
                            # queue (squares/divides) is never blocked by cc
                            nc.sync.dma_start(k_all[:, :, TQ:T].bitcast(F32),
                                              cc_out[peer_sp, :, :, 0:512])
                            nc.sync.dma_start(
                                v_all[:, NMCH:NMC, :, :].bitcast(F32).rearrange(
                                    "p a b c -> p a (b c)"),
                                cc_out[peer_sp, :, :, 512:1088])
                            nc.sync.dma_start(rk_all[:, NMCH:NMC, :],
                                              cc_out[peer_sp, :, :, 1088:CCW])

                    # ====== Q: q-proj + centered/scaled qa (PE broadcasts) ======
                    with (
                        tc.tile_pool(name="qw", bufs=2) as qw,
                        tc.tile_pool(name="qr", bufs=2) as qr,
                        tc.tile_pool(name="ps_qm", bufs=2, space="PSUM") as ps_qm,
                        tc.tile_pool(name="ps_qr", bufs=2, space="PSUM") as ps_qr_p,
                        tc.tile_pool(name="ps_qbc", bufs=2, space="PSUM") as ps_qbc,
                    ):
                        for hpr in range(4):          # pairs of head-pairs
                            rows = []
                            for hp in (2 * hpr, 2 * hpr + 1):
                                wq_sb = qw.tile([P, CH, P], F32R, name="wq_sb",
                                                tag="wq")
                                nc.scalar.dma_start(
                                    wq_sb[:], WprojT_r[:, :, hp * P:(hp + 1) * P])
                                nsq8 = qr.tile([65, TQ], F32, name="nsq8", tag="nsq8")
                                rq = qr.tile([65, TQ], F32R, name="rq", tag="rq")
                                for nt in range(NNT):
                                    nsl = slice(nt * 512, (nt + 1) * 512)
                                    ps_q = ps_qm.tile([P, 512], F32, name="ps_q",
                                                      tag="pm")
                                    for ch in range(CH):
                                        nc.tensor.matmul(ps_q[:], wq_sb[:, ch, :],
                                                         h_sb[:, ch, nsl],
                                                         start=(ch == 0),
                                                         stop=(ch == CH - 1))
                                    nc.vector.tensor_copy(qa_all[:, hp, nsl], ps_q[:])
                                    qsq = qr.tile([P, 512], BF16, name="qsq", tag="qsq")
                                    nc.gpsimd.tensor_tensor(qsq[:], qa_all[:, hp, nsl],
                                                            qa_all[:, hp, nsl], MUL)
                                    ps_qr = ps_qr_p.tile([65, 512], F32, name="ps_qr",
                                                         tag="pr")
                                    nc.tensor.matmul(ps_qr[:], bon8b[:],
                                                     qa_all[:, hp, nsl],
                                                     start=True, stop=True)
                                    ps_qr2 = ps_qr_p.tile([65, 512], F32, name="ps_qr2",
                                                          tag="pr")
                                    nc.tensor.matmul(ps_qr2[:], bo64b[:], qsq[:],
                                                     start=True, stop=True)
                                    nc.vector.tensor_copy(nsq8[:, nsl], ps_qr[:])
                                    # var slice = E[q^2] - mu^2 (rq finished below)
                                    m2q = qr.tile([65, 512], F32, name="m2q",
                                                  tag="m2q")
                                    nc.vector.tensor_scalar_mul(m2q[:], ps_qr[:],
                                                                -0.125)
                                    nc.vector.tensor_tensor(m2q[:], m2q[:], m2q[:],
                                                            MUL)
                                    nc.vector.scalar_tensor_tensor(rq[:, nsl],
                                                                   ps_qr2[:], 1.0,
                                                                   m2q[:], MUL, SUB)
                                # rq = exp(-0.5 ln(var+eps)); numr = mu*rq
                                nc.scalar.activation(rq[:], rq[:], LN_,
                                                     bias=eps_t[0:65], scale=1.0)
                                nc.scalar.activation(rq[:], rq[:], EXP,
                                                     bias=0.0, scale=-0.5)
                                numr = qr.tile([65, TQ], F32R, name="numr", tag="numr")
                                nc.vector.scalar_tensor_tensor(numr[:], nsq8[:], -0.125,
                                                               rq[:], MUL, MUL)
                                rows.append((hp, rq, numr))
                            # PE broadcast of rq/numr rows; qa in place
                            for hp, rq, numr in rows:
                                for quant, alu in ((rq, MUL), (numr, SUB)):
                                    bc = ps_qbc.tile([P, TQ], F32, name="bc",
                                                     tag="qbc")
                                    for nt in range(NNT):
                                        nsl = slice(nt * 512, (nt + 1) * 512)
                                        nc.tensor.matmul(bc[:, nsl], bsel[:],
                                                         quant[:, nsl],
                                                         start=True, stop=True)
                                    nc.vector.tensor_tensor(qa_all[:, hp, :],
                                                            qa_all[:, hp, :],
                                                            bc[:], alu)

                # h + A/Q PSUM pools freed here
                with tc.tile_pool(name="ofm", bufs=1) as ofm:
                    o_fm = ofm.tile([P, CH, TQ], F32R, name="o_fm")

                    with (
                        tc.tile_pool(name="psb", bufs=3) as psb,
                        tc.tile_pool(name="dsbp", bufs=2) as dsbp,
                        tc.tile_pool(name="denb", bufs=2) as denb,
                        tc.tile_pool(name="ps_att", bufs=2, space="PSUM") as ps_att,
                        tc.tile_pool(name="ps_av", bufs=1, space="PSUM") as ps_av,
                    ):
                        def att_pass(hp, mcs):
                            """scores+exp+av over key-chunks mcs for head-pair hp."""
                            av = ps_av.tile([65, 2, NNT, 512], F32, name="av",
                                            tag="av")
                            for i, mc in enumerate(mcs):
                                first, last = i == 0, i == len(mcs) - 1
                                for hh in range(2):
                                    g = 2 * hp + hh
                                    b0 = hh * 64
                                    sc = ps_att.tile([P, NNT, 512], F32, name="sc",
                                                     tag="sc")
                                    for nt in range(NNT):
                                        nc.tensor.matmul(
                                            sc[:, nt, :],
                                            k_all[b0:b0 + 64, hp, mc * P:(mc + 1) * P],
                                            qa_all[b0:b0 + 64, hp,
                                                   nt * 512:(nt + 1) * 512],
                                            start=True, stop=True)
                                    p_sb = psb.tile([P, NNT, 512], BF16, name="p_sb",
                                                    tag="p")
                                    nc.scalar.activation(p_sb[:], sc[:], EXP, bias=0.0,
                                                         scale=rk_all[:, mc, g:g + 1])
                                    for nt in range(NNT):
                                        nc.tensor.matmul(av[:, hh, nt, :],
                                                         v_all[:, mc, g, 0:65],
                                                         p_sb[:, nt, :],
                                                         start=first, stop=last)
                            return av

                        # ---- pass 1: own key chunks (overlaps the exchange) ----
                        for hp in range(CH):
                            av = att_pass(hp, range(NMCH))
                            dsb = dsbp.tile([65, 2048], F32, name="dsb", tag="dsb")
                            nc.vector.tensor_copy(
                                dsb[64:65, :],
                                av[64:65, :, :, :].rearrange("a b c d -> a (b c d)"))
                            nc.gpsimd.dma_start(den_d[hp:hp + 1, :], dsb[64:65, :])
                            for hh in range(2):
                                g = 2 * hp + hh
                                nc.vector.tensor_copy(
                                    o_fm[(g % 2) * 64:(g % 2) * 64 + 64, g // 2, :],
                                    av[0:64, hh, :, :].rearrange("a b c -> a (b c)"))

                        # ---- pass 2: peer key chunks + combine/divide ----
                        for hp in range(CH):
                            av = att_pass(hp, range(NMCH, NMC))
                            dsb = dsbp.tile([65, 2048], F32, name="dsb", tag="dsb")
                            nc.gpsimd.dma_start(dsb[64:65, :], den_d[hp:hp + 1, :])
                            nc.vector.tensor_tensor(
                                dsb[64:65, :], dsb[64:65, :],
                                av[64:65, :, :, :].rearrange("a b c d -> a (b c d)"),
                                ADD)
                            # rec = exp(-ln(den)) (single ln/exp table set)
                            nc.scalar.activation(dsb[64:65, :], dsb[64:65, :], LN_,
                                                 bias=eps_t[64:65], scale=1.0)
                            nc.scalar.activation(dsb[64:65, :], dsb[64:65, :], EXP,
                                                 bias=0.0, scale=-1.0)
                            nc.gpsimd.dma_start(den_d[hp:hp + 1, :], dsb[64:65, :])
                            den_bc = denb.tile([P, TQ], F32, name="den_bc", tag="den")
                            nc.sync.dma_start(
                                den_bc[0:64, :],
                                den_d[hp:hp + 1, 0:TQ].to_broadcast((64, TQ)))
                            nc.sync.dma_start(
                                den_bc[64:128, :],
                                den_d[hp:hp + 1, TQ:2048].to_broadcast((64, TQ)))
                            for hh in range(2):
                                g = 2 * hp + hh
                                osl = o_fm[(g % 2) * 64:(g % 2) * 64 + 64, g // 2, :]
                                nc.vector.tensor_tensor(
                                    osl, osl,
                                    av[0:64, hh, :, :].rearrange("a b c -> a (b c)"),
                                    ADD)
                                nc.gpsimd.tensor_tensor(
                                    osl, osl, den_bc[hh * 64:(hh + 1) * 64, :], MUL)

                    # ====== C: out = o_fm.T @ (I + W_out).T ======
                    with (
                        tc.tile_pool(name="cw", bufs=2) as cw,
                        tc.tile_pool(name="ps_c", bufs=3, space="PSUM") as ps_c,
                    ):
                        for jt in range(C // 512):
                            wres_sb = cw.tile([P, CH, 512], F32R, name="wres_sb",
                                              tag="wres")
                            nc.sync.dma_start(
                                wres_sb[:],
                                WresT_r[:, :, jt * 512:(jt + 1) * 512])
                            for ns in range(TQ // P):
                                ps_f = ps_c.tile([P, 512], F32, name="ps_f", tag="pc")
                                for cg in range(CH):
                                    nc.tensor.matmul(
                                        ps_f[:], o_fm[:, cg, ns * P:(ns + 1) * P],
                                        wres_sb[:, cg, :],
                                        start=(cg == 0), stop=(cg == CH - 1))
                                f_sb = cw.tile([P, 512], F32, name="f_sb", tag="fsb")
                                nc.vector.tensor_copy(f_sb[:], ps_f[:])
                                nc.gpsimd.dma_start(
                                    out[ns * P:(ns + 1) * P,
                                        jt * 512:(jt + 1) * 512],
                                    f_sb[:])

    nc.finalize()
    return nc


def _prep_host(x, emb, W_emb, b_emb, W_proj, W_out):
    W_embT = np.ascontiguousarray(W_emb.T.astype(np.float32))
    W_projT = np.ascontiguousarray(W_proj.T.astype(np.float32))
    W_resT = np.ascontiguousarray(
        (np.eye(C, dtype=np.float32) + W_out).T.astype(np.float32))
    bemb2 = b_emb.astype(np.float32).copy()
    bemb2[:C] += 1.0                       # fold the FiLM "+1" into the bias
    bemb_col = np.ascontiguousarray(bemb2.reshape(O2 // P, P).T)

    in_maps = []
    for c in range(NCORES):
        b, j = c // 2, c % 2
        tsl = slice(j * TQ, (j + 1) * TQ)
        in_maps.append({
            "xT": np.ascontiguousarray(x[b, tsl].T.astype(np.float32)),
            "embT": np.ascontiguousarray(emb[b, tsl].T.astype(np.float32)),
            "WembT": W_embT, "bemb": bemb_col,
            "WprojT": W_projT, "WresT": W_resT,
        })
    return in_maps


def kernel(x, emb, W_emb, b_emb, W_proj, W_out, _trace=False):
    x = np.asarray(x); emb = np.asarray(emb)
    W_emb = np.asarray(W_emb); b_emb = np.asarray(b_emb)
    W_proj = np.asarray(W_proj); W_out = np.asarray(W_out)

    if "nc" not in _cached:
        _cached["nc"] = build_kernel()
    nc = _cached["nc"]

    in_maps = _prep_host(x, emb, W_emb, b_emb, W_proj, W_out)
    res = run_bass_kernel_spmd(nc, in_maps, core_ids=list(range(NCORES)),
                               trace=_trace)
    _cached["last_result"] = res

    outp = np.empty((B, N, C), dtype=np.float32)
    for c in range(NCORES):
        b, j = c // 2, c % 2
        outp[b, j * TQ:(j + 1) * TQ, :] = res.results[c]["out"]
    return outp


if __name__ == "__main__":
    build_kernel()
    print("BUILD OK")
